# revision 1
# baseline (speedup 1.0000x reference)
"""Trainium2 Bass kernel for nn_Attention_24343874633732.

Full multi-head attention (RoPE variant + GQA + additive mask + out-proj),
B=4, S=1024, D=2048, H=32 q-heads, 8 kv-heads, head_dim 64, fp32 in/out.

Sharding: 8 cores = 4 (batch, data parallel) x 2 (head groups, tensor
parallel: wq/wk/wv output dim and wo input dim split in half). Each core
computes a partial (S, D) output for one batch element; the host sums the
two TP partials per batch element.

Host-side simplifications baked into the per-core inputs:
  - The reference's RoPE indexes the cos/sin tables by *head index* (not
    position), so the rotation is a per-head constant linear map folded into
    wq/wk on the host. The 1/sqrt(head_dim) score scale is folded into wq.
  - The mask is asserted to be the standard causal 0/-inf mask. Per key
    block kb and query stripe qc only the causally-live query window
    (F = 512 - max(0, 128*kb - 512*qc) columns) is computed; the single
    diagonal 128x128 sub-block is masked by multiplying with one shared
    [k <= q] indicator tile (P^T = exp(S^T) * triu).
  - Heads are permuted so each q head occupies the SBUF partition half that
    matches its kv head's half; score matmuls (contraction K=64) then run
    as lane-disjoint pairs on the PE array.
  - All matmul inputs are cast to bf16 on the host (fp32 PSUM accumulation
    on device); weights are pre-packed so every load is one large
    contiguous DMA.

Device pipeline per core (S^T layout, no on-device transposes):
  QT = per-pair matmuls -> (qdim, seq); KT -> (kvdim, seq); V -> (seq,
  kvdim) with a ones column appended per kv head. Per head pair, per query
  stripe: for each live key block, S^T = KT_h.T @ QT_h on the live query
  window (per-head windows bank-aligned in PSUM); P^T = exp(S^T) [* triu on
  the diagonal block]; [O^T; l] += V'_h.T @ P^T (the ones column yields the
  softmax denominator l for free). Normalization is deferred into the next
  stripe so it never stalls the PE: 1/l via DVE reciprocal straight off the
  PSUM l rows, partition-broadcast with one block-diagonal-ones matmul,
  ActE stages O^T to SBUF (DVE may read only one PSUM operand) and DVE
  scales it in place. O^T stays resident in SBUF; the output projection
  runs last against prefetched woT tiles and streams fp32 partials out.
  The next pair's Q-projection chain drains as PE filler inside the
  ActE-dense attention stripes, and x/wk/wv stream in seq/kv halves so the
  first stripe starts as early as possible.
"""

import os

import numpy as np

import concourse.bass as bass
import concourse.mybir as mybir
import concourse.tile as tile
from concourse.bass_utils import run_bass_kernel_spmd
from concourse.vector_clock import ScopedClock

H, KV, HD = 32, 8, 64
B, S, D = 4, 1024, 2048
NH = 16  # q heads per core
NKV = 4  # kv heads per core
QD = NH * HD  # 1024, per-core q projection dim
KD = NKV * HD  # 256, per-core kv projection dim
NKC = D // 128  # 16 contraction chunks for projections
NSEQ = S // 128  # 8 seq chunks
NQC = 2  # q stripes of 512 in attention
NKB = S // 128  # 8 key blocks of 128

F32 = mybir.dt.float32
EXPF = mybir.ActivationFunctionType.Exp

# local head order: position p holds local head LOCAL_ORDER[p]; even
# positions hold heads whose local kv index is even (partition half 0),
# odd positions kv-odd heads (half 1). Pairs (2j, 2j+1) share a KT tile.
LOCAL_ORDER = [0, 4, 1, 5, 2, 6, 3, 7, 8, 12, 9, 13, 10, 14, 11, 15]

_last_perf = {}
_module_cache = {}


class SplitDrainTileContext(tile.TileContext):
    """TileContext whose final drain carries at most one sync wait.

    The pinned walrus rejects CTRL/NOP instructions with more than one sync
    wait; excess waits move onto dedicated single-wait NOPs.
    """

    def _drain_and_barrier(self, tick_clock, wait_clock):
        nc = self.nc
        drain_inst = nc.sync.drain()
        wait_clock.add_sem_waits(
            drain_inst.ins, ScopedClock({None: tick_clock.global_clock})
        )
        si = drain_inst.ins.sync_info
        waits = list(si.on_wait or [])
        if len(waits) > 1:
            drain_inst.ins.sync_info = mybir.SyncInfo(
                on_wait=[waits[0]], on_update=list(si.on_update or [])
            )
            for w in waits[1:]:
                nop = nc.sync.nop(nofuse=True)
                nop.ins.sync_info = mybir.SyncInfo(on_wait=[w], on_update=[])
        nc.all_engine_barrier()
        assert self.sems is not None
        popped = nc._tile_sem_poison_stack.pop()
        assert popped is self._sem_poison
        nc.clear_and_free_semaphores(list(self.sems.allocated().values()))
        nc.all_engine_barrier()


def _mm_dt():
    return {
        "bf16": mybir.dt.bfloat16,
        "f32r": mybir.dt.float32r,
        "f32": mybir.dt.float32,
    }[os.environ.get("KERNEL_MM_DT", "bf16")]


# per-instruction-struct sync-wait capacity of the pinned walrus; waits
# beyond the limit are hoisted onto single-wait NOPs on the same engine
# (engine order preserved, so gating semantics are unchanged)
_WAIT_LIMITS = {}
_DEFAULT_WAIT_LIMIT = 1


def _split_excess_waits(nc):
    blocks = [b for f in nc.m.functions for b in f.blocks]
    need = {}
    for blk in blocks:
        for inst in blk.instructions:
            si = getattr(inst, "sync_info", None)
            if not si or not si.on_wait:
                continue
            lim = _WAIT_LIMITS.get(type(inst).__name__, _DEFAULT_WAIT_LIMIT)
            n = len(si.on_wait)
            if n > lim:
                need[inst.engine] = need.get(inst.engine, 0) + (n - lim)
    if not need:
        return
    spares = {}
    spare_names = set()
    for eng, cnt in need.items():
        engine = nc.engines[eng]
        lst = []
        for _ in range(cnt):
            bi = engine.nop(nofuse=True)
            lst.append(bi.ins)
            spare_names.add(bi.ins.name)
        spares[eng] = lst
    for blk in blocks:
        il = blk.instructions
        if any(i.name in spare_names for i in il):
            blk.instructions = [i for i in il if i.name not in spare_names]
    for blk in blocks:
        il = list(blk.instructions)
        out = []
        changed = False
        for inst in il:
            si = getattr(inst, "sync_info", None)
            waits = list(si.on_wait) if si and si.on_wait else []
            lim = _WAIT_LIMITS.get(type(inst).__name__, _DEFAULT_WAIT_LIMIT)
            if len(waits) > lim:
                changed = True
                for w in waits[lim:]:
                    nop = spares[inst.engine].pop()
                    nop.sync_info = mybir.SyncInfo(on_wait=[w], on_update=[])
                    out.append(nop)
                inst.sync_info = mybir.SyncInfo(
                    on_wait=waits[:lim], on_update=list(si.on_update or [])
                )
            out.append(inst)
        if changed:
            blk.instructions = out


def _win(qc, kb):
    """Live query window start (within the 512 stripe) for key block kb."""
    return max(0, 128 * kb - 512 * qc)


def build_module():
    """Build the per-core Bass module (causal mask structure hardcoded)."""
    from contextlib import ExitStack

    mdt = _mm_dt()

    nc = bass.Bass()
    xT_d = nc.dram_tensor("xT", [D, S], mdt, kind="ExternalInput")
    wqq_d = nc.dram_tensor("wqq", [8, 128, NKC, 128], mdt, kind="ExternalInput")
    wkh_d = nc.dram_tensor("wkh", [2, 128, NKC, 128], mdt, kind="ExternalInput")
    wvh_d = nc.dram_tensor("wvh", [2, 128, NKC, 128], mdt, kind="ExternalInput")
    woT_d = nc.dram_tensor("woT", [QD, D], mdt, kind="ExternalInput")
    em_d = nc.dram_tensor("emTril", [128, 128], mdt, kind="ExternalInput")
    onesb_d = nc.dram_tensor("ones_bd", [33, 128], mybir.dt.float32r, kind="ExternalInput")
    out_d = nc.dram_tensor("out", [S, D], F32, kind="ExternalOutput")

    LOOK = 2
    with SplitDrainTileContext(nc) as tc, ExitStack() as top:
        persist = top.enter_context(tc.tile_pool(name="persist", bufs=1))
        qtp = top.enter_context(tc.tile_pool(name="qtp", bufs=3))
        ptp = top.enter_context(tc.tile_pool(name="pt", bufs=5))
        psq = top.enter_context(tc.tile_pool(name="psq", bufs=2, space="PSUM"))
        pssc = top.enter_context(tc.tile_pool(name="pssc", bufs=2, space="PSUM"))
        pvs = top.enter_context(tc.tile_pool(name="pvs", bufs=2, space="PSUM"))

        kt = [persist.tile([128, S], mdt, tag=f"kt{i}", name=f"kt{i}") for i in range(2)]
        vp = [persist.tile([128, NKV, HD + 1], mdt, tag=f"vp{i}", name=f"vp{i}") for i in range(8)]
        ot = [persist.tile([128, S], mdt, tag=f"ot{i}", name=f"ot{i}") for i in range(8)]
        em = persist.tile([128, 128], mdt, tag="em", name="em")
        onesb = persist.tile([33, 128], mybir.dt.float32r, tag="onesb", name="onesb")
        rpb = [
            persist.tile([33, 512], mybir.dt.float32r, tag=f"rpb{i}", name=f"rpb{i}")
            for i in range(2)
        ]
        wot = [
            persist.tile([128, D], mdt, tag=f"wot{i}", name=f"wot{i}")
            for i in range(8)
        ]

        # ---------------- projections + attention, interleaved ----------
        with ExitStack() as ph1:
            wkvp = ph1.enter_context(tc.tile_pool(name="wkv", bufs=1))
            xtp = ph1.enter_context(tc.tile_pool(name="xt", bufs=1))
            wqqp = ph1.enter_context(tc.tile_pool(name="wqq", bufs=2))

            # staged input tiles; x is one tile loaded in grouped seq-half
            # DMAs (streaming granularity without per-tile min-transfer
            # floors); wk/wv split into kv-halves (packed on the host) so
            # the first attention only waits on the half it needs
            xta = xtp.tile([128, NKC, S], mdt, tag="xta", name="xta")
            xt = [xta[:, kc, :] for kc in range(NKC)]
            wk2 = [
                wkvp.tile([128, NKC, 128], mdt, tag=f"wk{g}", name=f"wk{g}")
                for g in range(2)
            ]
            wv2 = [
                wkvp.tile([128, NKC, 128], mdt, tag=f"wv{g}", name=f"wv{g}")
                for g in range(2)
            ]

            def dma_wq(j, wt):
                nc.sync.dma_start(out=wt[:], in_=wqq_d[j])

            def dma_wkv(g):
                nc.sync.dma_start(out=wk2[g][:], in_=wkh_d[g])
                nc.sync.dma_start(out=wv2[g][:], in_=wvh_d[g])

            # startup stream, seq-half granular: the first attention stripe
            # (q and k in [0,512)) needs only the x seq-half 0, the first
            # wk/wv kv-halves, and wq(0) — stream those first so scores
            # start ~6us earlier; x seq-half 1 streams during the first
            # stripe
            wt0 = wqqp.tile([128, NKC, 128], mdt, tag="wqq", name="wqq")
            xT_v = xT_d.rearrange("(k p) s -> p k s", p=128)
            nc.sync.dma_start(
                out=xta[:, 0:2, 0:512], in_=xT_v[:, 0:2, 0:512]
            )
            nc.sync.dma_start(out=wk2[0][:, 0:4, :], in_=wkh_d[0, :, 0:4, :])
            nc.sync.dma_start(
                out=xta[:, 2:4, 0:512], in_=xT_v[:, 2:4, 0:512]
            )
            nc.sync.dma_start(out=wk2[0][:, 4:NKC, :], in_=wkh_d[0, :, 4:NKC, :])
            nc.sync.dma_start(
                out=xta[:, 4:8, 0:512], in_=xT_v[:, 4:8, 0:512]
            )
            dma_wq(0, wt0)
            for g in range(2, 4):
                nc.sync.dma_start(
                    out=xta[:, 4 * g : 4 * (g + 1), 0:512],
                    in_=xT_v[:, 4 * g : 4 * (g + 1), 0:512],
                )
            nc.sync.dma_start(out=em[:], in_=em_d[:, :])
            nc.sync.dma_start(out=onesb[:], in_=onesb_d[:, :])
            nc.sync.dma_start(out=wv2[0][:], in_=wvh_d[0])
            for g in range(4):
                nc.sync.dma_start(
                    out=xta[:, 4 * g : 4 * (g + 1), 512:1024],
                    in_=xT_v[:, 4 * g : 4 * (g + 1), 512:1024],
                )
            # V ones columns via tiny memsets, off the DMA queue
            for sm in range(8):
                nc.vector.memset(vp[sm][:, :, HD : HD + 1], 1.0)
            # zero the rp middle partitions once: the broadcast matmul's
            # zero stationary rows must not meet NaN garbage on hardware
            for i in range(2):
                nc.vector.memset(rpb[i][0:32, :].bitcast(F32), 0.0)

            # K projection half-chain -> kt[m2] seq-half n
            def emit_k_half(m2, n):
                ps = psq.tile([128, 512], F32, tag="psq", name="psq")
                for kc in range(NKC):
                    nc.tensor.matmul(
                        ps[:],
                        wk2[m2][:, kc, :],
                        xt[kc][:, 512 * n : 512 * (n + 1)],
                        start=(kc == 0),
                        stop=(kc == NKC - 1),
                    )
                nc.vector.tensor_copy(kt[m2][:, 512 * n : 512 * (n + 1)], ps[:])

            # V projection kv-half -> vp[sm][:, 2g:2g+2] (seq-part layout)
            def emit_v_half(sm, g):
                ps = psq.tile([128, 512], F32, tag="psq", name="psq")
                for kc in range(NKC):
                    nc.tensor.matmul(
                        ps[:, 0:128],
                        xt[kc][:, 128 * sm : 128 * (sm + 1)],
                        wv2[g][:, kc, :],
                        start=(kc == 0),
                        stop=(kc == NKC - 1),
                    )
                nc.vector.tensor_copy(
                    vp[sm][:, 2 * g : 2 * g + 2, 0:HD],
                    ps[:, 0:128].rearrange("p (g d) -> p g d", g=2),
                )

            # deferred Q projection: returns the qtj tile plus a list of
            # single-matmul thunks to be drained as PE filler inside the
            # ActE-dense attention stripes
            def deferred_qt(j, wt):
                qtj = qtp.tile([128, S], mdt, tag="qt", name="qt")
                state = {}
                thunks = []
                for n in range(2):
                    for kc in range(NKC):
                        def th(n=n, kc=kc):
                            if kc == 0:
                                state[n] = psq.tile(
                                    [128, 512], F32, tag="psq", name="psq"
                                )
                            nc.tensor.matmul(
                                state[n][:],
                                wt[:, kc, :],
                                xt[kc][:, 512 * n : 512 * (n + 1)],
                                start=(kc == 0),
                                stop=(kc == NKC - 1),
                            )
                            if kc == NKC - 1:
                                # split the two chain copies across ActE/DVE
                                # to balance the attention-phase load
                                if n == 0:
                                    nc.scalar.activation(
                                        qtj[:, 0:512],
                                        state[n][:],
                                        mybir.ActivationFunctionType.Copy,
                                    )
                                else:
                                    nc.vector.tensor_copy(
                                        qtj[:, 512:1024], state[n][:]
                                    )
                        thunks.append(th)
                return qtj, thunks

            fq = []

            def drain(k):
                for _ in range(min(k, len(fq))):
                    fq.pop(0)()

            def drain_all():
                while fq:
                    fq.pop(0)()

            def emit_norm_recip(j, qc, pvp):
                """1/l rows for the previous stripe, emitted at the NEXT
                stripe's entry so they run on DVE before its mask-muls."""
                rp = rpb[(2 * j + qc) % 2]
                for h in range(2):
                    with nc.allow_low_precision(reason="f32r is fp32-width"):
                        nc.vector.reciprocal(
                            rp[32 * h : 32 * h + 1, :], pvp[h][HD : HD + 1, :]
                        )
                return rp

            def emit_norm(j, qc, pvp, rp=None):
                """Deferred softmax normalize for (j, qc): partition-broadcast
                of the 1/l rows with a ones matmul, multiply fused into the
                PSUM->SBUF copy into resident O^T. Emitted a couple of score
                steps into the NEXT stripe so it never stalls the PE."""
                if rp is None:
                    rp = emit_norm_recip(j, qc, pvp)
                # one block-diag-ones matmul broadcasts both heads' 1/l rows;
                # the stationary's zero rows null the uninitialized middle
                # partitions of rp
                bc = psq.tile([128, 512], F32, tag="psq", name="psq")
                nc.tensor.matmul(bc[:], onesb[:], rp[:], start=True, stop=True)
                for h in range(2):
                    dst = ot[j][64 * h : 64 * h + 64, 512 * qc : 512 * (qc + 1)]
                    # DVE may read only one PSUM operand: stage O^T to SBUF
                    # first, then scale in place against the PSUM-resident
                    # broadcast rows (both on DVE, which has more slack than
                    # the exp-loaded ActE during attention)
                    nc.vector.tensor_copy(dst, pvp[h][0:HD, :])
                    nc.vector.tensor_mul(dst, dst, bc[64 * h : 64 * h + 64, :])

            def emit_att(j, qtj, qc, pending):
                """Scores+exp+mask+PV for (j, qc); returns (j, qc, pvp) for
                deferred normalization. `pending` is the previous stripe's
                deferral, emitted after this stripe's first LOOK steps."""
                ktj = kt[j // 4]
                kv_even = 2 * (j // 4)
                kbl = [kb for kb in range(NKB) if 128 * kb < 512 * (qc + 1)]
                pvp = [
                    pvs.tile([HD + 1, 512], F32, tag="pvs", name="pvs")
                    for _ in range(2)
                ]
                rp_pend = (
                    emit_norm_recip(*pending) if pending is not None else None
                )
                pts = {}
                for step in range(len(kbl) + LOOK):
                    if step < len(kbl):
                        kb = kbl[step]
                        w = _win(qc, kb)
                        F = 512 - w
                        ps = pssc.tile([128, 1024], F32, tag="pssc", name="pssc")
                        for h in range(2):
                            # per-head windows bank-aligned at 512h (a matmul
                            # output may not cross a PSUM bank boundary)
                            nc.tensor.matmul(
                                ps[:, 512 * h : 512 * h + F],
                                ktj[64 * h : 64 * h + 64, 128 * kb : 128 * (kb + 1)],
                                qtj[64 * h : 64 * h + 64, 512 * qc + w : 512 * (qc + 1)],
                                start=True,
                                stop=True,
                            )
                        pt = ptp.tile([128, 1024], mdt, tag="pt", name="pt")
                        if F == 512:
                            nc.scalar.activation(pt[:], ps[:], EXPF)
                        else:
                            nc.scalar.activation(
                                pt[:].rearrange("p (t q) -> p t q", t=2)[:, :, 0:F],
                                ps[:].rearrange("p (t q) -> p t q", t=2)[:, :, 0:F],
                                EXPF,
                            )
                        if 128 * kb >= 512 * qc:
                            # diagonal sub-block: first 128 cols of window
                            for h in range(2):
                                nc.vector.tensor_mul(
                                    pt[:, 512 * h : 512 * h + 128],
                                    pt[:, 512 * h : 512 * h + 128],
                                    em[:],
                                )
                        pts[kb] = (pt, w, F)
                    if step == LOOK and pending is not None:
                        emit_norm(*pending, rp=rp_pend)
                        pending = None
                    if step >= LOOK:
                        kb = kbl[step - LOOK]
                        first = step - LOOK == 0
                        last = step - LOOK == len(kbl) - 1
                        pt, w, F = pts.pop(kb)
                        for h in range(2):
                            nc.tensor.matmul(
                                pvp[h][:, w:512],
                                vp[kb][:, kv_even + h, :],
                                pt[:, 512 * h : 512 * h + F],
                                start=first,
                                stop=last,
                            )
                    drain(3)
                if pending is not None:
                    emit_norm(*pending)
                return (j, qc, pvp)

            # emission order: only what attention j0 needs goes first, so
            # the ScalarE exp stream starts as early as possible
            # startup: K/Q seq-half-0 chains interleaved per x-chunk so the
            # PE tracks the x DMA stream; the V chains, remaining halves,
            # and the NEXT pair's Q chain drain as filler inside the
            # ActE-dense attention stripes
            qtj0, qthunks0 = deferred_qt(0, wt0)
            psK = psq.tile([128, 512], F32, tag="psq", name="psq")
            for kc in range(NKC):
                nc.tensor.matmul(
                    psK[:],
                    wk2[0][:, kc, :],
                    xt[kc][:, 0:512],
                    start=(kc == 0),
                    stop=(kc == NKC - 1),
                )
            nc.vector.tensor_copy(kt[0][:, 0:512], psK[:])
            for kc in range(NKC):
                qthunks0[kc]()
            fq.extend([lambda sm=sm: emit_v_half(sm, 0) for sm in range(4)])
            fq.append(lambda: emit_k_half(0, 1))
            fq.extend(qthunks0[NKC:])

            wot_order = list(range(8))
            qt_cur = qtj0
            pending = None
            for j in range(8):
                if j < 7:
                    wt = wqqp.tile([128, NKC, 128], mdt, tag="wqq", name="wqq")
                    dma_wq(j + 1, wt)
                    qt_next, qthunks = deferred_qt(j + 1, wt)
                    fq.extend(qthunks)
                if j == 0:
                    # second kv-halves of wk/wv, behind the wq(1) chunks
                    dma_wkv(1)
                if j == 1:
                    # pair-4+ prerequisites: kt[1] and the vp kv-half 1
                    fq.extend([lambda n=n: emit_k_half(1, n) for n in range(2)])
                    fq.extend(
                        [lambda sm=sm: emit_v_half(sm, 1) for sm in range(NSEQ)]
                    )
                for _ in range(2 if j == 1 else (1 if j >= 2 else 0)):
                    if wot_order:
                        i = wot_order.pop(0)
                        nc.sync.dma_start(
                            out=wot[i][:], in_=woT_d[128 * i : 128 * (i + 1), :]
                        )
                pending = emit_att(j, qt_cur, 0, pending)
                if j == 0:
                    drain_all()
                    for sm in range(4, NSEQ):
                        emit_v_half(sm, 0)
                pending = emit_att(j, qt_cur, 1, pending)
                drain_all()
                if j < 7:
                    qt_cur = qt_next
            while wot_order:
                i = wot_order.pop(0)
                nc.sync.dma_start(
                    out=wot[i][:], in_=woT_d[128 * i : 128 * (i + 1), :]
                )
            final_norm = pending

        # ---------------- output projection ----------------
        with ExitStack() as ph3:
            outp = ph3.enter_context(tc.tile_pool(name="outsb", bufs=5))
            for dnp in range(2):
                for sm in range(NSEQ):
                    # the last stripe's normalize rides behind the first two
                    # sm chains (which only read qc=0 columns of O^T)
                    if dnp == 0 and sm == 2 and final_norm is not None:
                        emit_norm(*final_norm)
                        final_norm = None
                    # quarter-granular sub-chains only on the very last tile,
                    # so the post-matmul copy+store drain tail is short
                    parts = (
                        [(0, 512), (512, 256), (768, 256)]
                        if (dnp == 1 and sm == NSEQ - 1)
                        else [(0, 512), (512, 512)]
                    )
                    # alternate tiles between the pssc and (otherwise idle)
                    # psq pools so chains never wait on a single pool's
                    # rotation; psq tiles are [128,512] so odd-sm halves each
                    # get their own tile
                    use_psq = sm % 2 == 1 and not (dnp == 1 and sm == NSEQ - 1)
                    if not use_psq:
                        ps = pssc.tile([128, 1024], F32, tag="pssc", name="pssc")
                    for off, fw in parts:
                        if use_psq:
                            half = psq.tile([128, 512], F32, tag="psq", name="psq")
                            dst_ps, dst_off = half, 0
                        else:
                            dst_ps, dst_off = ps, off
                        for qd in range(8):
                            nc.tensor.matmul(
                                dst_ps[:, dst_off : dst_off + fw],
                                ot[qd][:, 128 * sm : 128 * (sm + 1)],
                                wot[qd][
                                    :, 1024 * dnp + off : 1024 * dnp + off + fw
                                ],
                                start=(qd == 0),
                                stop=(qd == 7),
                            )
                        # copy+store each part as soon as its chain stops, so
                        # the drain tail is one part, not a full tile
                        ob = outp.tile([128, 512], F32, tag="outsb", name="outsb")
                        nc.scalar.activation(
                            ob[:, 0:fw],
                            dst_ps[:, dst_off : dst_off + fw],
                            mybir.ActivationFunctionType.Copy,
                        )
                        nc.sync.dma_start(
                            out=out_d[
                                128 * sm : 128 * (sm + 1),
                                1024 * dnp + off : 1024 * dnp + off + fw,
                            ],
                            in_=ob[:, 0:fw],
                        )

    _split_excess_waits(nc)
    nc.finalize()
    return nc


# ---------------------------------------------------------------------------
# host-side preparation
# ---------------------------------------------------------------------------


def _fold_rope(w, cos, sin, nh, scale):
    c = cos[:nh].astype(np.float64)
    s = sin[:nh].astype(np.float64)
    wr = w.astype(np.float64).reshape(nh, HD // 2, 2, w.shape[-1])
    o0 = c[:, :, None] * wr[:, :, 0] - s[:, :, None] * wr[:, :, 1]
    o1 = s[:, :, None] * wr[:, :, 0] + c[:, :, None] * wr[:, :, 1]
    return (np.stack([o0, o1], axis=2).reshape(w.shape) * scale).astype(np.float32)


def _np_dt():
    return mybir.dt.np(_mm_dt())


def kernel(x, freqs_cos, freqs_sin, mask, wq, wk, wv, wo):
    x = np.asarray(x, dtype=np.float32)
    freqs_cos = np.asarray(freqs_cos, dtype=np.float32)
    freqs_sin = np.asarray(freqs_sin, dtype=np.float32)
    mask = np.asarray(mask, dtype=np.float32)
    wq = np.asarray(wq, dtype=np.float32)
    wk = np.asarray(wk, dtype=np.float32)
    wv = np.asarray(wv, dtype=np.float32)
    wo = np.asarray(wo, dtype=np.float32)

    # the kernel hardcodes the causal structure; verify it holds
    causal = np.where(
        np.tril(np.ones((S, S), dtype=bool)), 0.0, -np.inf
    ).astype(np.float32)
    assert np.array_equal(mask, causal), "kernel specialized to causal mask"

    wq_rot = _fold_rope(wq, freqs_cos, freqs_sin, H, 1.0 / np.sqrt(HD))
    wk_rot = _fold_rope(wk, freqs_cos, freqs_sin, KV, 1.0)

    ndt = _np_dt()
    key = os.environ.get("KERNEL_MM_DT", "bf16")
    nc = _module_cache.get(key)
    if nc is None:
        nc = build_module()
        _module_cache[key] = nc

    # S^T layout: tile[k, q] keeps k <= q, i.e. upper-triangular
    em_tril = np.triu(np.ones((128, 128), np.float32)).astype(ndt)
    ones_bd = np.zeros((33, 128), np.float32)
    ones_bd[0, 0:64] = 1.0
    ones_bd[32, 64:128] = 1.0

    in_maps = []
    for c in range(8):
        b, t = divmod(c, 2)
        order = [16 * t + p for p in LOCAL_ORDER]
        kv_heads = list(range(4 * t, 4 * t + 4))
        wq_c = wq_rot.reshape(H, HD, D)[order].reshape(QD, D)
        wk_c = wk_rot.reshape(KV, HD, D)[kv_heads].reshape(KD, D)
        wv_c = wv.reshape(KV, HD, D)[kv_heads].reshape(KD, D)
        wo_c = wo.reshape(D, H, HD)[:, order].reshape(D, QD)
        # packed weight layouts: [chunk-of-128-outputs, 128 D-partitions,
        # NKC D-chunks, 128 outputs], contiguous per chunk for 1-DMA loads
        def pack(wT, nchunk):
            return (
                wT.reshape(NKC, 128, 128 * nchunk)
                .transpose(1, 0, 2)
                .reshape(128, NKC, nchunk, 128)
                .transpose(2, 0, 1, 3)
            )

        wqq = pack(wq_c.T, 8)  # (8, 128, NKC, 128)
        wkh = pack(wk_c.T, 2)  # (2, 128, NKC, 128)
        wvh = pack(wv_c.T, 2)
        in_maps.append(
            {
                "xT": np.ascontiguousarray(x[b].T).astype(ndt),
                "wqq": np.ascontiguousarray(wqq).astype(ndt),
                "wkh": np.ascontiguousarray(wkh).astype(ndt),
                "wvh": np.ascontiguousarray(wvh).astype(ndt),
                "woT": np.ascontiguousarray(wo_c.T).astype(ndt),
                "emTril": em_tril,
                "ones_bd": ones_bd,
            }
        )

    trace = bool(os.environ.get("KERNEL_TRACE"))
    res = run_bass_kernel_spmd(nc, in_maps, core_ids=list(range(8)), trace=trace)
    _last_perf["exec_time_ns"] = res.exec_time_ns
    _last_perf["mean_exec_time_ns"] = res.mean_exec_time_ns
    _last_perf["results"] = res

    out = np.empty((B, S, D), np.float32)
    for b in range(B):
        out[b] = res.results[2 * b]["out"] + res.results[2 * b + 1]["out"]
    return out



# revision 6
# speedup vs baseline: 1.3702x; 1.3702x over previous
"""Trainium2 Bass kernel for nn_Attention_24343874633732.

Full multi-head attention (RoPE variant + GQA + additive mask + out-proj),
B=4, S=1024, D=2048, H=32 q-heads, 8 kv-heads, head_dim 64, fp32 in/out.

Sharding: 8 cores = 4 (batch, data parallel) x 2 (head groups, tensor
parallel: wq/wk/wv output dim and wo input dim split in half). Each core
computes a partial (S, D) output for one batch element; the host sums the
two TP partials per batch element.

Host-side simplifications baked into the per-core inputs:
  - The reference's RoPE indexes the cos/sin tables by *head index* (not
    position), so the rotation is a per-head constant linear map folded into
    wq/wk on the host. The 1/sqrt(head_dim) score scale is folded into wq.
  - The mask is asserted to be the standard causal 0/-inf mask. Per key
    block kb and query stripe qc only the causally-live query window
    (F = 512 - max(0, 128*kb - 512*qc) columns) is computed; the single
    diagonal 128x128 sub-block is masked by multiplying with one shared
    [k <= q] indicator tile (P^T = exp(S^T) * triu).
  - Heads are permuted so each q head occupies the SBUF partition half that
    matches its kv head's half; score matmuls (contraction K=64) then run
    as lane-disjoint pairs on the PE array.
  - All matmul inputs are cast to bf16 on the host (fp32 PSUM accumulation
    on device); weights are pre-packed so every load is one large
    contiguous DMA.

Device pipeline per core (S^T layout, no on-device transposes):
  QT = per-pair matmuls -> (qdim, seq); KT -> (kvdim, seq); V -> (seq,
  kvdim) with a ones column appended per kv head. Per head pair, per query
  stripe: for each live key block, S^T = KT_h.T @ QT_h on the live query
  window (per-head windows bank-aligned in PSUM); P^T = exp(S^T) [* triu on
  the diagonal block]; [O^T; l] += V'_h.T @ P^T (the ones column yields the
  softmax denominator l for free). Normalization is deferred into the next
  stripe so it never stalls the PE: 1/l via DVE reciprocal straight off the
  PSUM l rows, partition-broadcast with one block-diagonal-ones matmul,
  ActE stages O^T to SBUF (DVE may read only one PSUM operand) and DVE
  scales it in place. O^T stays resident in SBUF; the output projection
  runs last against prefetched woT tiles and streams fp32 partials out.
  The next pair's Q-projection chain drains as PE filler inside the
  ActE-dense attention stripes, and x/wk/wv stream in seq/kv halves so the
  first stripe starts as early as possible.
"""

import os

import numpy as np

import concourse.bass as bass
import concourse.mybir as mybir
import concourse.tile as tile
from concourse.bass_utils import run_bass_kernel_spmd
from concourse.vector_clock import ScopedClock

H, KV, HD = 32, 8, 64
B, S, D = 4, 1024, 2048
NH = 16  # q heads per core
NKV = 4  # kv heads per core
QD = NH * HD  # 1024, per-core q projection dim
KD = NKV * HD  # 256, per-core kv projection dim
NKC = D // 128  # 16 contraction chunks for projections
NSEQ = S // 128  # 8 seq chunks
NQC = 2  # q stripes of 512 in attention
NKB = S // 128  # 8 key blocks of 128

F32 = mybir.dt.float32
EXPF = mybir.ActivationFunctionType.Exp

# local head order: position p holds local head LOCAL_ORDER[p]; even
# positions hold heads whose local kv index is even (partition half 0),
# odd positions kv-odd heads (half 1). Pairs (2j, 2j+1) share a KT tile.
LOCAL_ORDER = [0, 4, 1, 5, 2, 6, 3, 7, 8, 12, 9, 13, 10, 14, 11, 15]

_last_perf = {}
_module_cache = {}


class SplitDrainTileContext(tile.TileContext):
    """TileContext whose final drain carries at most one sync wait.

    The pinned walrus rejects CTRL/NOP instructions with more than one sync
    wait; excess waits move onto dedicated single-wait NOPs.
    """

    def _drain_and_barrier(self, tick_clock, wait_clock):
        nc = self.nc
        drain_inst = nc.sync.drain()
        wait_clock.add_sem_waits(
            drain_inst.ins, ScopedClock({None: tick_clock.global_clock})
        )
        si = drain_inst.ins.sync_info
        waits = list(si.on_wait or [])
        if len(waits) > 1:
            drain_inst.ins.sync_info = mybir.SyncInfo(
                on_wait=[waits[0]], on_update=list(si.on_update or [])
            )
            for w in waits[1:]:
                nop = nc.sync.nop(nofuse=True)
                nop.ins.sync_info = mybir.SyncInfo(on_wait=[w], on_update=[])
        nc.all_engine_barrier()
        assert self.sems is not None
        popped = nc._tile_sem_poison_stack.pop()
        assert popped is self._sem_poison
        nc.clear_and_free_semaphores(list(self.sems.allocated().values()))
        nc.all_engine_barrier()


def _mm_dt():
    return {
        "bf16": mybir.dt.bfloat16,
        "f32r": mybir.dt.float32r,
        "f32": mybir.dt.float32,
    }[os.environ.get("KERNEL_MM_DT", "bf16")]


# per-instruction-struct sync-wait capacity of the pinned walrus; waits
# beyond the limit are hoisted onto single-wait NOPs on the same engine
# (engine order preserved, so gating semantics are unchanged)
_WAIT_LIMITS = {}
_DEFAULT_WAIT_LIMIT = 1


def _split_excess_waits(nc):
    blocks = [b for f in nc.m.functions for b in f.blocks]
    need = {}
    for blk in blocks:
        for inst in blk.instructions:
            si = getattr(inst, "sync_info", None)
            if not si or not si.on_wait:
                continue
            lim = _WAIT_LIMITS.get(type(inst).__name__, _DEFAULT_WAIT_LIMIT)
            n = len(si.on_wait)
            if n > lim:
                need[inst.engine] = need.get(inst.engine, 0) + (n - lim)
    if not need:
        return
    spares = {}
    spare_names = set()
    for eng, cnt in need.items():
        engine = nc.engines[eng]
        lst = []
        for _ in range(cnt):
            bi = engine.nop(nofuse=True)
            lst.append(bi.ins)
            spare_names.add(bi.ins.name)
        spares[eng] = lst
    for blk in blocks:
        il = blk.instructions
        if any(i.name in spare_names for i in il):
            blk.instructions = [i for i in il if i.name not in spare_names]
    for blk in blocks:
        il = list(blk.instructions)
        out = []
        changed = False
        for inst in il:
            si = getattr(inst, "sync_info", None)
            waits = list(si.on_wait) if si and si.on_wait else []
            lim = _WAIT_LIMITS.get(type(inst).__name__, _DEFAULT_WAIT_LIMIT)
            if len(waits) > lim:
                changed = True
                for w in waits[lim:]:
                    nop = spares[inst.engine].pop()
                    nop.sync_info = mybir.SyncInfo(on_wait=[w], on_update=[])
                    out.append(nop)
                inst.sync_info = mybir.SyncInfo(
                    on_wait=waits[:lim], on_update=list(si.on_update or [])
                )
            out.append(inst)
        if changed:
            blk.instructions = out


def _win(qc, kb):
    """Live query window start (within the 512 stripe) for key block kb."""
    return max(0, 128 * kb - 512 * qc)


def build_module():
    """Build the per-core Bass module (causal mask structure hardcoded)."""
    from contextlib import ExitStack

    mdt = _mm_dt()

    nc = bass.Bass()
    xT_d = nc.dram_tensor("xT", [D, S], mdt, kind="ExternalInput")
    wqq_d = nc.dram_tensor("wqq", [8, 128, NKC, 128], mdt, kind="ExternalInput")
    wkh_d = nc.dram_tensor("wkh", [2, 128, NKC, 128], mdt, kind="ExternalInput")
    wvh_d = nc.dram_tensor("wvh", [2, 128, NKC, 128], mdt, kind="ExternalInput")
    woT_d = nc.dram_tensor("woT", [QD, D], mdt, kind="ExternalInput")
    em_d = nc.dram_tensor("emTril", [128, 128], mdt, kind="ExternalInput")
    onesb_d = nc.dram_tensor("ones_bd", [33, 128], mybir.dt.float32r, kind="ExternalInput")
    out_d = nc.dram_tensor("out", [S, D], F32, kind="ExternalOutput")

    LOOK = 2
    with SplitDrainTileContext(nc) as tc, ExitStack() as top:
        persist = top.enter_context(tc.tile_pool(name="persist", bufs=1))
        qtp = top.enter_context(tc.tile_pool(name="qtp", bufs=3))
        ptp = top.enter_context(tc.tile_pool(name="pt", bufs=5))
        bcsp = top.enter_context(tc.tile_pool(name="bcs", bufs=2))
        psq = top.enter_context(tc.tile_pool(name="psq", bufs=2, space="PSUM"))
        pssc = top.enter_context(tc.tile_pool(name="pssc", bufs=2, space="PSUM"))
        pvs = top.enter_context(tc.tile_pool(name="pvs", bufs=2, space="PSUM"))

        kt = [persist.tile([128, S], mdt, tag=f"kt{i}", name=f"kt{i}") for i in range(2)]
        vp = [persist.tile([128, NKV, HD + 1], mdt, tag=f"vp{i}", name=f"vp{i}") for i in range(8)]
        ot = [persist.tile([128, S], mdt, tag=f"ot{i}", name=f"ot{i}") for i in range(8)]
        em = persist.tile([128, 128], mdt, tag="em", name="em")
        onesb = persist.tile([33, 128], mybir.dt.float32r, tag="onesb", name="onesb")
        rpb = [
            persist.tile([33, 512], mybir.dt.float32r, tag=f"rpb{i}", name=f"rpb{i}")
            for i in range(2)
        ]
        wot = [
            persist.tile([128, D], mdt, tag=f"wot{i}", name=f"wot{i}")
            for i in range(8)
        ]

        # ---------------- projections + attention, interleaved ----------
        with ExitStack() as ph1:
            wkvp = ph1.enter_context(tc.tile_pool(name="wkv", bufs=1))
            xtp = ph1.enter_context(tc.tile_pool(name="xt", bufs=1))
            wqqp = ph1.enter_context(tc.tile_pool(name="wqq", bufs=2))

            # staged input tiles; x is one tile loaded in grouped seq-half
            # DMAs (streaming granularity without per-tile min-transfer
            # floors); wk/wv split into kv-halves (packed on the host) so
            # the first attention only waits on the half it needs
            xta = xtp.tile([128, NKC, S], mdt, tag="xta", name="xta")
            xt = [xta[:, kc, :] for kc in range(NKC)]
            wk2 = [
                wkvp.tile([128, NKC, 128], mdt, tag=f"wk{g}", name=f"wk{g}")
                for g in range(2)
            ]
            wv2 = [
                wkvp.tile([128, NKC, 128], mdt, tag=f"wv{g}", name=f"wv{g}")
                for g in range(2)
            ]

            def dma_wq(j, wt):
                nc.sync.dma_start(out=wt[:], in_=wqq_d[j])

            def dma_wkv(g):
                nc.sync.dma_start(out=wk2[g][:], in_=wkh_d[g])
                nc.sync.dma_start(out=wv2[g][:], in_=wvh_d[g])

            # startup stream, seq-half granular: the first attention stripe
            # (q and k in [0,512)) needs only the x seq-half 0, the first
            # wk/wv kv-halves, and wq(0) — stream those first so scores
            # start ~6us earlier; x seq-half 1 streams during the first
            # stripe
            wt0 = wqqp.tile([128, NKC, 128], mdt, tag="wqq", name="wqq")
            xT_v = xT_d.rearrange("(k p) s -> p k s", p=128)
            nc.sync.dma_start(
                out=xta[:, 0:2, 0:512], in_=xT_v[:, 0:2, 0:512]
            )
            nc.sync.dma_start(out=wk2[0][:, 0:4, :], in_=wkh_d[0, :, 0:4, :])
            nc.sync.dma_start(
                out=xta[:, 2:4, 0:512], in_=xT_v[:, 2:4, 0:512]
            )
            nc.sync.dma_start(out=wk2[0][:, 4:NKC, :], in_=wkh_d[0, :, 4:NKC, :])
            nc.sync.dma_start(
                out=xta[:, 4:8, 0:512], in_=xT_v[:, 4:8, 0:512]
            )
            dma_wq(0, wt0)
            for g in range(2, 4):
                nc.sync.dma_start(
                    out=xta[:, 4 * g : 4 * (g + 1), 0:512],
                    in_=xT_v[:, 4 * g : 4 * (g + 1), 0:512],
                )
            nc.sync.dma_start(out=em[:], in_=em_d[:, :])
            nc.sync.dma_start(out=onesb[:], in_=onesb_d[:, :])
            nc.sync.dma_start(out=wv2[0][:], in_=wvh_d[0])
            for g in range(4):
                nc.sync.dma_start(
                    out=xta[:, 4 * g : 4 * (g + 1), 512:1024],
                    in_=xT_v[:, 4 * g : 4 * (g + 1), 512:1024],
                )
            # V ones columns via tiny memsets, off the DMA queue
            for sm in range(8):
                nc.vector.memset(vp[sm][:, :, HD : HD + 1], 1.0)
            # zero the rp middle partitions once: the broadcast matmul's
            # zero stationary rows must not meet NaN garbage on hardware
            for i in range(2):
                nc.vector.memset(rpb[i][0:32, :].bitcast(F32), 0.0)

            # K projection half-chain -> kt[m2] seq-half n
            def emit_k_half(m2, n):
                ps = psq.tile([128, 512], F32, tag="psq", name="psq")
                for kc in range(NKC):
                    nc.tensor.matmul(
                        ps[:],
                        wk2[m2][:, kc, :],
                        xt[kc][:, 512 * n : 512 * (n + 1)],
                        start=(kc == 0),
                        stop=(kc == NKC - 1),
                    )
                nc.vector.tensor_copy(kt[m2][:, 512 * n : 512 * (n + 1)], ps[:])

            # V projection kv-half -> vp[sm][:, 2g:2g+2] (seq-part layout)
            def emit_v_half(sm, g):
                ps = psq.tile([128, 512], F32, tag="psq", name="psq")
                for kc in range(NKC):
                    nc.tensor.matmul(
                        ps[:, 0:128],
                        xt[kc][:, 128 * sm : 128 * (sm + 1)],
                        wv2[g][:, kc, :],
                        start=(kc == 0),
                        stop=(kc == NKC - 1),
                    )
                nc.vector.tensor_copy(
                    vp[sm][:, 2 * g : 2 * g + 2, 0:HD],
                    ps[:, 0:128].rearrange("p (g d) -> p g d", g=2),
                )

            # deferred Q projection: returns the qtj tile plus a list of
            # single-matmul thunks to be drained as PE filler inside the
            # ActE-dense attention stripes
            def deferred_qt(j, wt):
                qtj = qtp.tile([128, S], mdt, tag="qt", name="qt")
                state = {}
                thunks = []
                for n in range(2):
                    for kc in range(NKC):
                        def th(n=n, kc=kc):
                            if kc == 0:
                                state[n] = psq.tile(
                                    [128, 512], F32, tag="psq", name="psq"
                                )
                            nc.tensor.matmul(
                                state[n][:],
                                wt[:, kc, :],
                                xt[kc][:, 512 * n : 512 * (n + 1)],
                                start=(kc == 0),
                                stop=(kc == NKC - 1),
                            )
                            if kc == NKC - 1:
                                # split the two chain copies across ActE/DVE
                                # to balance the attention-phase load
                                if n == 0:
                                    nc.scalar.activation(
                                        qtj[:, 0:512],
                                        state[n][:],
                                        mybir.ActivationFunctionType.Copy,
                                    )
                                else:
                                    nc.vector.tensor_copy(
                                        qtj[:, 512:1024], state[n][:]
                                    )
                        thunks.append(th)
                return qtj, thunks

            fq = []

            def drain(k):
                for _ in range(min(k, len(fq))):
                    fq.pop(0)()

            def drain_all():
                while fq:
                    fq.pop(0)()

            def emit_norm_recip(j, qc, pvp):
                """ln(l) rows for the previous stripe on ActE. DVE's
                RECIPROCAL is an iterative 8-cycle/element op (3.4us per
                row) that was stalling the PE at every stripe boundary;
                ln/exp live in one act table set (natural_log_exp_and_
                others) so 1/l = exp(-ln(l)) runs at 1 elem/cycle with no
                table switch."""
                rp = rpb[(2 * j + qc) % 2]
                for h in range(2):
                    with nc.allow_low_precision(reason="f32r is fp32-width"):
                        nc.scalar.activation(
                            rp[32 * h : 32 * h + 1, :],
                            pvp[h][HD : HD + 1, :],
                            mybir.ActivationFunctionType.Ln,
                        )
                return rp

            def emit_norm(j, qc, pvp, rp=None, copies_done=False):
                """Deferred softmax normalize for (j, qc): partition-broadcast
                of the ln(l) rows with a ones matmul, exp(-x) on ActE turns
                the broadcast into 1/l, DVE scales resident O^T. Emitted a
                few score steps into the NEXT stripe so it never stalls the
                PE."""
                if rp is None:
                    rp = emit_norm_recip(j, qc, pvp)
                # one block-diag-ones matmul broadcasts both heads' ln(l)
                # rows; the stationary's zero rows null the uninitialized
                # middle partitions of rp
                bc = psq.tile([128, 512], F32, tag="psq", name="psq")
                nc.tensor.matmul(bc[:], onesb[:], rp[:], start=True, stop=True)
                bcs = bcsp.tile([128, 512], F32, tag="bcs", name="bcs")
                nc.scalar.activation(bcs[:], bc[:], EXPF, scale=-1.0)
                for h in range(2):
                    dst = ot[j][64 * h : 64 * h + 64, 512 * qc : 512 * (qc + 1)]
                    if not copies_done:
                        nc.vector.tensor_copy(dst, pvp[h][0:HD, :])
                    nc.vector.tensor_mul(dst, dst, bcs[64 * h : 64 * h + 64, :])

            def emit_att(j, qtj, qc, pending):
                """Scores+exp+mask+PV for (j, qc); returns (j, qc, pvp) for
                deferred normalization. `pending` is the previous stripe's
                deferral: its O^T copies + ln(l) run behind this stripe's
                first exp (freeing its PSUM), the broadcast+scale a couple
                of steps later."""
                ktj = kt[j // 4]
                kv_even = 2 * (j // 4)
                kbl = [kb for kb in range(NKB) if 128 * kb < 512 * (qc + 1)]
                pvp = [
                    pvs.tile([HD + 1, 512], F32, tag="pvs", name="pvs")
                    for _ in range(2)
                ]
                rp_pend = None
                pts = {}
                for step in range(len(kbl) + LOOK):
                    if step < len(kbl):
                        kb = kbl[step]
                        w = _win(qc, kb)
                        F = 512 - w
                        ps = pssc.tile([128, 1024], F32, tag="pssc", name="pssc")
                        for h in range(2):
                            # per-head windows bank-aligned at 512h (a matmul
                            # output may not cross a PSUM bank boundary)
                            nc.tensor.matmul(
                                ps[:, 512 * h : 512 * h + F],
                                ktj[64 * h : 64 * h + 64, 128 * kb : 128 * (kb + 1)],
                                qtj[64 * h : 64 * h + 64, 512 * qc + w : 512 * (qc + 1)],
                                start=True,
                                stop=True,
                            )
                        pt = ptp.tile([128, 1024], mdt, tag="pt", name="pt")
                        if F == 512:
                            nc.scalar.activation(pt[:], ps[:], EXPF)
                        else:
                            nc.scalar.activation(
                                pt[:].rearrange("p (t q) -> p t q", t=2)[:, :, 0:F],
                                ps[:].rearrange("p (t q) -> p t q", t=2)[:, :, 0:F],
                                EXPF,
                            )
                        if 128 * kb >= 512 * qc:
                            # diagonal sub-block: first 128 cols of window
                            for h in range(2):
                                nc.vector.tensor_mul(
                                    pt[:, 512 * h : 512 * h + 128],
                                    pt[:, 512 * h : 512 * h + 128],
                                    em[:],
                                )
                        pts[kb] = (pt, w, F)
                    if step == 0 and pending is not None:
                        # stage the previous stripe's O^T to SBUF now (frees
                        # its PV PSUM tiles before this stripe's chains need
                        # the pool) and queue ln(l) behind this stripe's
                        # first exp on ActE
                        pj, pqc, ppvp = pending
                        for h in range(2):
                            nc.vector.tensor_copy(
                                ot[pj][64 * h : 64 * h + 64, 512 * pqc : 512 * (pqc + 1)],
                                ppvp[h][0:HD, :],
                            )
                        rp_pend = emit_norm_recip(pj, pqc, ppvp)
                    if step == LOOK + 1 and pending is not None:
                        emit_norm(*pending, rp=rp_pend, copies_done=True)
                        pending = None
                    if step >= LOOK:
                        kb = kbl[step - LOOK]
                        first = step - LOOK == 0
                        last = step - LOOK == len(kbl) - 1
                        pt, w, F = pts.pop(kb)
                        for h in range(2):
                            nc.tensor.matmul(
                                pvp[h][:, w:512],
                                vp[kb][:, kv_even + h, :],
                                pt[:, 512 * h : 512 * h + F],
                                start=first,
                                stop=last,
                            )
                    drain(3)
                if pending is not None:
                    emit_norm(*pending)
                return (j, qc, pvp)

            # emission order: only what attention j0 needs goes first, so
            # the ScalarE exp stream starts as early as possible
            # startup: K/Q seq-half-0 chains interleaved per x-chunk so the
            # PE tracks the x DMA stream; the V chains, remaining halves,
            # and the NEXT pair's Q chain drain as filler inside the
            # ActE-dense attention stripes
            qtj0, qthunks0 = deferred_qt(0, wt0)
            psK = psq.tile([128, 512], F32, tag="psq", name="psq")
            for kc in range(NKC):
                nc.tensor.matmul(
                    psK[:],
                    wk2[0][:, kc, :],
                    xt[kc][:, 0:512],
                    start=(kc == 0),
                    stop=(kc == NKC - 1),
                )
            nc.vector.tensor_copy(kt[0][:, 0:512], psK[:])
            for kc in range(NKC):
                qthunks0[kc]()
            fq.extend([lambda sm=sm: emit_v_half(sm, 0) for sm in range(4)])
            fq.append(lambda: emit_k_half(0, 1))
            fq.extend(qthunks0[NKC:])

            wot_order = list(range(8))
            qt_cur = qtj0
            pending = None
            for j in range(8):
                if j < 7:
                    wt = wqqp.tile([128, NKC, 128], mdt, tag="wqq", name="wqq")
                    dma_wq(j + 1, wt)
                    qt_next, qthunks = deferred_qt(j + 1, wt)
                    fq.extend(qthunks)
                if j == 0:
                    # second kv-halves of wk/wv, behind the wq(1) chunks
                    dma_wkv(1)
                if j == 1:
                    # pair-4+ prerequisites: kt[1] and the vp kv-half 1
                    fq.extend([lambda n=n: emit_k_half(1, n) for n in range(2)])
                    fq.extend(
                        [lambda sm=sm: emit_v_half(sm, 1) for sm in range(NSEQ)]
                    )
                for _ in range(2 if j == 1 else (1 if j >= 2 else 0)):
                    if wot_order:
                        i = wot_order.pop(0)
                        nc.sync.dma_start(
                            out=wot[i][:], in_=woT_d[128 * i : 128 * (i + 1), :]
                        )
                pending = emit_att(j, qt_cur, 0, pending)
                if j == 0:
                    drain_all()
                    for sm in range(4, NSEQ):
                        emit_v_half(sm, 0)
                pending = emit_att(j, qt_cur, 1, pending)
                drain_all()
                if j < 7:
                    qt_cur = qt_next
            while wot_order:
                i = wot_order.pop(0)
                nc.sync.dma_start(
                    out=wot[i][:], in_=woT_d[128 * i : 128 * (i + 1), :]
                )
            final_norm = pending

        # ---------------- output projection ----------------
        with ExitStack() as ph3:
            outp = ph3.enter_context(tc.tile_pool(name="outsb", bufs=5))
            for dnp in range(2):
                for sm in range(NSEQ):
                    # the last stripe's normalize rides behind the first two
                    # sm chains (which only read qc=0 columns of O^T)
                    if dnp == 0 and sm == 2 and final_norm is not None:
                        emit_norm(*final_norm)
                        final_norm = None
                    # quarter-granular sub-chains only on the very last tile,
                    # so the post-matmul copy+store drain tail is short
                    parts = (
                        [(0, 512), (512, 256), (768, 256)]
                        if (dnp == 1 and sm == NSEQ - 1)
                        else [(0, 512), (512, 512)]
                    )
                    # alternate tiles between the pssc and (otherwise idle)
                    # psq pools so chains never wait on a single pool's
                    # rotation; psq tiles are [128,512] so odd-sm halves each
                    # get their own tile
                    use_psq = sm % 2 == 1 and not (dnp == 1 and sm == NSEQ - 1)
                    if not use_psq:
                        ps = pssc.tile([128, 1024], F32, tag="pssc", name="pssc")
                    for off, fw in parts:
                        if use_psq:
                            half = psq.tile([128, 512], F32, tag="psq", name="psq")
                            dst_ps, dst_off = half, 0
                        else:
                            dst_ps, dst_off = ps, off
                        for qd in range(8):
                            nc.tensor.matmul(
                                dst_ps[:, dst_off : dst_off + fw],
                                ot[qd][:, 128 * sm : 128 * (sm + 1)],
                                wot[qd][
                                    :, 1024 * dnp + off : 1024 * dnp + off + fw
                                ],
                                start=(qd == 0),
                                stop=(qd == 7),
                            )
                        # copy+store each part as soon as its chain stops, so
                        # the drain tail is one part, not a full tile
                        ob = outp.tile([128, 512], F32, tag="outsb", name="outsb")
                        nc.scalar.activation(
                            ob[:, 0:fw],
                            dst_ps[:, dst_off : dst_off + fw],
                            mybir.ActivationFunctionType.Copy,
                        )
                        nc.sync.dma_start(
                            out=out_d[
                                128 * sm : 128 * (sm + 1),
                                1024 * dnp + off : 1024 * dnp + off + fw,
                            ],
                            in_=ob[:, 0:fw],
                        )

    _split_excess_waits(nc)
    nc.finalize()
    return nc


# ---------------------------------------------------------------------------
# host-side preparation
# ---------------------------------------------------------------------------


def _fold_rope(w, cos, sin, nh, scale):
    c = cos[:nh].astype(np.float64)
    s = sin[:nh].astype(np.float64)
    wr = w.astype(np.float64).reshape(nh, HD // 2, 2, w.shape[-1])
    o0 = c[:, :, None] * wr[:, :, 0] - s[:, :, None] * wr[:, :, 1]
    o1 = s[:, :, None] * wr[:, :, 0] + c[:, :, None] * wr[:, :, 1]
    return (np.stack([o0, o1], axis=2).reshape(w.shape) * scale).astype(np.float32)


def _np_dt():
    return mybir.dt.np(_mm_dt())


def kernel(x, freqs_cos, freqs_sin, mask, wq, wk, wv, wo):
    x = np.asarray(x, dtype=np.float32)
    freqs_cos = np.asarray(freqs_cos, dtype=np.float32)
    freqs_sin = np.asarray(freqs_sin, dtype=np.float32)
    mask = np.asarray(mask, dtype=np.float32)
    wq = np.asarray(wq, dtype=np.float32)
    wk = np.asarray(wk, dtype=np.float32)
    wv = np.asarray(wv, dtype=np.float32)
    wo = np.asarray(wo, dtype=np.float32)

    # the kernel hardcodes the causal structure; verify it holds
    causal = np.where(
        np.tril(np.ones((S, S), dtype=bool)), 0.0, -np.inf
    ).astype(np.float32)
    assert np.array_equal(mask, causal), "kernel specialized to causal mask"

    wq_rot = _fold_rope(wq, freqs_cos, freqs_sin, H, 1.0 / np.sqrt(HD))
    wk_rot = _fold_rope(wk, freqs_cos, freqs_sin, KV, 1.0)

    ndt = _np_dt()
    key = os.environ.get("KERNEL_MM_DT", "bf16")
    nc = _module_cache.get(key)
    if nc is None:
        nc = build_module()
        _module_cache[key] = nc

    # S^T layout: tile[k, q] keeps k <= q, i.e. upper-triangular
    em_tril = np.triu(np.ones((128, 128), np.float32)).astype(ndt)
    ones_bd = np.zeros((33, 128), np.float32)
    ones_bd[0, 0:64] = 1.0
    ones_bd[32, 64:128] = 1.0

    in_maps = []
    for c in range(8):
        b, t = divmod(c, 2)
        order = [16 * t + p for p in LOCAL_ORDER]
        kv_heads = list(range(4 * t, 4 * t + 4))
        wq_c = wq_rot.reshape(H, HD, D)[order].reshape(QD, D)
        wk_c = wk_rot.reshape(KV, HD, D)[kv_heads].reshape(KD, D)
        wv_c = wv.reshape(KV, HD, D)[kv_heads].reshape(KD, D)
        wo_c = wo.reshape(D, H, HD)[:, order].reshape(D, QD)
        # packed weight layouts: [chunk-of-128-outputs, 128 D-partitions,
        # NKC D-chunks, 128 outputs], contiguous per chunk for 1-DMA loads
        def pack(wT, nchunk):
            return (
                wT.reshape(NKC, 128, 128 * nchunk)
                .transpose(1, 0, 2)
                .reshape(128, NKC, nchunk, 128)
                .transpose(2, 0, 1, 3)
            )

        wqq = pack(wq_c.T, 8)  # (8, 128, NKC, 128)
        wkh = pack(wk_c.T, 2)  # (2, 128, NKC, 128)
        wvh = pack(wv_c.T, 2)
        in_maps.append(
            {
                "xT": np.ascontiguousarray(x[b].T).astype(ndt),
                "wqq": np.ascontiguousarray(wqq).astype(ndt),
                "wkh": np.ascontiguousarray(wkh).astype(ndt),
                "wvh": np.ascontiguousarray(wvh).astype(ndt),
                "woT": np.ascontiguousarray(wo_c.T).astype(ndt),
                "emTril": em_tril,
                "ones_bd": ones_bd,
            }
        )

    trace = bool(os.environ.get("KERNEL_TRACE"))
    res = run_bass_kernel_spmd(nc, in_maps, core_ids=list(range(8)), trace=trace)
    _last_perf["exec_time_ns"] = res.exec_time_ns
    _last_perf["mean_exec_time_ns"] = res.mean_exec_time_ns
    _last_perf["results"] = res

    out = np.empty((B, S, D), np.float32)
    for b in range(B):
        out[b] = res.results[2 * b]["out"] + res.results[2 * b + 1]["out"]
    return out



# revision 15
# speedup vs baseline: 1.3743x; 1.0030x over previous
"""Trainium2 Bass kernel for nn_Attention_24343874633732.

Full multi-head attention (RoPE variant + GQA + additive mask + out-proj),
B=4, S=1024, D=2048, H=32 q-heads, 8 kv-heads, head_dim 64, fp32 in/out.

Sharding: 8 cores = 4 (batch, data parallel) x 2 (head groups, tensor
parallel: wq/wk/wv output dim and wo input dim split in half). Each core
computes a partial (S, D) output for one batch element; the host sums the
two TP partials per batch element.

Host-side simplifications baked into the per-core inputs:
  - The reference's RoPE indexes the cos/sin tables by *head index* (not
    position), so the rotation is a per-head constant linear map folded into
    wq/wk on the host. The 1/sqrt(head_dim) score scale is folded into wq.
  - The mask is asserted to be the standard causal 0/-inf mask. Per key
    block kb and query stripe qc only the causally-live query window
    (F = 512 - max(0, 128*kb - 512*qc) columns) is computed; the single
    diagonal 128x128 sub-block is masked by multiplying with one shared
    [k <= q] indicator tile (P^T = exp(S^T) * triu).
  - Heads are permuted so each q head occupies the SBUF partition half that
    matches its kv head's half; score matmuls (contraction K=64) then run
    as lane-disjoint pairs on the PE array.
  - All matmul inputs are cast to bf16 on the host (fp32 PSUM accumulation
    on device); weights are pre-packed so every load is one large
    contiguous DMA.

Device pipeline per core (S^T layout, no on-device transposes):
  QT = per-pair matmuls -> (qdim, seq); KT -> (kvdim, seq); V -> (seq,
  kvdim) with a ones column appended per kv head. Per head pair, per query
  stripe: for each live key block, S^T = KT_h.T @ QT_h on the live query
  window (per-head windows bank-aligned in PSUM); P^T = exp(S^T) [* triu on
  the diagonal block]; [O^T; l] += V'_h.T @ P^T (the ones column yields the
  softmax denominator l for free). Normalization is deferred into the next
  stripe so it never stalls the PE: 1/l via DVE reciprocal straight off the
  PSUM l rows, partition-broadcast with one block-diagonal-ones matmul,
  ActE stages O^T to SBUF (DVE may read only one PSUM operand) and DVE
  scales it in place. O^T stays resident in SBUF; the output projection
  runs last against prefetched woT tiles and streams fp32 partials out.
  The next pair's Q-projection chain drains as PE filler inside the
  ActE-dense attention stripes, and x/wk/wv stream in seq/kv halves so the
  first stripe starts as early as possible.
"""

import os

import numpy as np

import concourse.bass as bass
import concourse.mybir as mybir
import concourse.tile as tile
from concourse.bass_utils import run_bass_kernel_spmd
from concourse.vector_clock import ScopedClock

H, KV, HD = 32, 8, 64
B, S, D = 4, 1024, 2048
NH = 16  # q heads per core
NKV = 4  # kv heads per core
QD = NH * HD  # 1024, per-core q projection dim
KD = NKV * HD  # 256, per-core kv projection dim
NKC = D // 128  # 16 contraction chunks for projections
NSEQ = S // 128  # 8 seq chunks
NQC = 2  # q stripes of 512 in attention
NKB = S // 128  # 8 key blocks of 128

F32 = mybir.dt.float32
EXPF = mybir.ActivationFunctionType.Exp

# local head order: position p holds local head LOCAL_ORDER[p]; even
# positions hold heads whose local kv index is even (partition half 0),
# odd positions kv-odd heads (half 1). Pairs (2j, 2j+1) share a KT tile.
LOCAL_ORDER = [0, 4, 1, 5, 2, 6, 3, 7, 8, 12, 9, 13, 10, 14, 11, 15]

_last_perf = {}
_module_cache = {}


class SplitDrainTileContext(tile.TileContext):
    """TileContext whose final drain carries at most one sync wait.

    The pinned walrus rejects CTRL/NOP instructions with more than one sync
    wait; excess waits move onto dedicated single-wait NOPs.
    """

    def _drain_and_barrier(self, tick_clock, wait_clock):
        nc = self.nc
        drain_inst = nc.sync.drain()
        wait_clock.add_sem_waits(
            drain_inst.ins, ScopedClock({None: tick_clock.global_clock})
        )
        si = drain_inst.ins.sync_info
        waits = list(si.on_wait or [])
        if len(waits) > 1:
            drain_inst.ins.sync_info = mybir.SyncInfo(
                on_wait=[waits[0]], on_update=list(si.on_update or [])
            )
            for w in waits[1:]:
                nop = nc.sync.nop(nofuse=True)
                nop.ins.sync_info = mybir.SyncInfo(on_wait=[w], on_update=[])
        nc.all_engine_barrier()
        assert self.sems is not None
        popped = nc._tile_sem_poison_stack.pop()
        assert popped is self._sem_poison
        nc.clear_and_free_semaphores(list(self.sems.allocated().values()))
        nc.all_engine_barrier()


def _mm_dt():
    return {
        "bf16": mybir.dt.bfloat16,
        "f32r": mybir.dt.float32r,
        "f32": mybir.dt.float32,
    }[os.environ.get("KERNEL_MM_DT", "bf16")]


# per-instruction-struct sync-wait capacity of the pinned walrus; waits
# beyond the limit are hoisted onto single-wait NOPs on the same engine
# (engine order preserved, so gating semantics are unchanged)
_WAIT_LIMITS = {}
_DEFAULT_WAIT_LIMIT = 1


def _split_excess_waits(nc):
    blocks = [b for f in nc.m.functions for b in f.blocks]
    need = {}
    for blk in blocks:
        for inst in blk.instructions:
            si = getattr(inst, "sync_info", None)
            if not si or not si.on_wait:
                continue
            lim = _WAIT_LIMITS.get(type(inst).__name__, _DEFAULT_WAIT_LIMIT)
            n = len(si.on_wait)
            if n > lim:
                need[inst.engine] = need.get(inst.engine, 0) + (n - lim)
    if not need:
        return
    spares = {}
    spare_names = set()
    for eng, cnt in need.items():
        engine = nc.engines[eng]
        lst = []
        for _ in range(cnt):
            bi = engine.nop(nofuse=True)
            lst.append(bi.ins)
            spare_names.add(bi.ins.name)
        spares[eng] = lst
    for blk in blocks:
        il = blk.instructions
        if any(i.name in spare_names for i in il):
            blk.instructions = [i for i in il if i.name not in spare_names]
    for blk in blocks:
        il = list(blk.instructions)
        out = []
        changed = False
        for inst in il:
            si = getattr(inst, "sync_info", None)
            waits = list(si.on_wait) if si and si.on_wait else []
            lim = _WAIT_LIMITS.get(type(inst).__name__, _DEFAULT_WAIT_LIMIT)
            if len(waits) > lim:
                changed = True
                for w in waits[lim:]:
                    nop = spares[inst.engine].pop()
                    nop.sync_info = mybir.SyncInfo(on_wait=[w], on_update=[])
                    out.append(nop)
                inst.sync_info = mybir.SyncInfo(
                    on_wait=waits[:lim], on_update=list(si.on_update or [])
                )
            out.append(inst)
        if changed:
            blk.instructions = out


def _win(qc, kb):
    """Live query window start (within the 512 stripe) for key block kb."""
    return max(0, 128 * kb - 512 * qc)


def build_module():
    """Build the per-core Bass module (causal mask structure hardcoded)."""
    from contextlib import ExitStack

    mdt = _mm_dt()

    nc = bass.Bass()
    # x packed seq-half-major on the host: [half, 128 D-partitions, NKC
    # D-chunks, 512 seq] with (chunk, seq) contiguous per partition, so
    # startup DMA descriptors are 2-8KB runs instead of 1KB
    xT_d = nc.dram_tensor("xTh", [2, 128, NKC, 512], mdt, kind="ExternalInput")
    wqq_d = nc.dram_tensor("wqq", [8, 128, NKC, 128], mdt, kind="ExternalInput")
    wkh_d = nc.dram_tensor("wkh", [2, 128, NKC, 128], mdt, kind="ExternalInput")
    wvh_d = nc.dram_tensor("wvh", [128, NKC, 2 * 128], mdt, kind="ExternalInput")
    woT_d = nc.dram_tensor("woT", [QD, D], mdt, kind="ExternalInput")
    em_d = nc.dram_tensor("emTril", [128, 128], mdt, kind="ExternalInput")
    onesb_d = nc.dram_tensor("ones_bd", [33, 128], mybir.dt.float32r, kind="ExternalInput")
    out_d = nc.dram_tensor("out", [S, D], F32, kind="ExternalOutput")

    LOOK = 2
    with SplitDrainTileContext(nc) as tc, ExitStack() as top:
        persist = top.enter_context(tc.tile_pool(name="persist", bufs=1))
        qtp = top.enter_context(tc.tile_pool(name="qtp", bufs=3))
        ptp = top.enter_context(tc.tile_pool(name="pt", bufs=5))
        bcsp = top.enter_context(tc.tile_pool(name="bcs", bufs=2))
        psq = top.enter_context(tc.tile_pool(name="psq", bufs=2, space="PSUM"))
        pssc = top.enter_context(tc.tile_pool(name="pssc", bufs=2, space="PSUM"))
        pvs = top.enter_context(tc.tile_pool(name="pvs", bufs=2, space="PSUM"))

        kt = [persist.tile([128, S], mdt, tag=f"kt{i}", name=f"kt{i}") for i in range(2)]
        vp = [persist.tile([128, NKV, HD + 1], mdt, tag=f"vp{i}", name=f"vp{i}") for i in range(8)]
        ot = [persist.tile([128, S], mdt, tag=f"ot{i}", name=f"ot{i}") for i in range(8)]
        em = persist.tile([128, 128], mdt, tag="em", name="em")
        onesb = persist.tile([33, 128], mybir.dt.float32r, tag="onesb", name="onesb")
        rpb = [
            persist.tile([33, 512], mybir.dt.float32r, tag=f"rpb{i}", name=f"rpb{i}")
            for i in range(2)
        ]
        wot = [
            persist.tile([128, D], mdt, tag=f"wot{i}", name=f"wot{i}")
            for i in range(8)
        ]

        # ---------------- projections + attention, interleaved ----------
        with ExitStack() as ph1:
            wkvp = ph1.enter_context(tc.tile_pool(name="wkv", bufs=1))
            xtp = ph1.enter_context(tc.tile_pool(name="xt", bufs=1))
            wqqp = ph1.enter_context(tc.tile_pool(name="wqq", bufs=2))

            # staged input tiles; x is one tile loaded in grouped seq-half
            # DMAs (streaming granularity without per-tile min-transfer
            # floors); wk/wv split into kv-halves (packed on the host) so
            # the first attention only waits on the half it needs
            xta = xtp.tile([128, NKC, S], mdt, tag="xta", name="xta")
            xt = [xta[:, kc, :] for kc in range(NKC)]
            wk2 = [
                wkvp.tile([128, NKC, 128], mdt, tag=f"wk{g}", name=f"wk{g}")
                for g in range(2)
            ]
            wvm = wkvp.tile([128, NKC, 2 * 128], mdt, tag="wvm", name="wvm")

            def dma_wq(j, wt):
                nc.sync.dma_start(out=wt[:], in_=wqq_d[j])

            # startup stream, seq-half granular: the first attention stripe
            # (q and k in [0,512)) needs only the x seq-half 0, the first
            # wk/wv kv-halves, and wq(0) — stream those first so scores
            # start ~6us earlier; x seq-half 1 streams during the first
            # stripe
            wt0 = wqqp.tile([128, NKC, 128], mdt, tag="wqq", name="wqq")
            nc.sync.dma_start(out=xta[:, 0:2, 0:512], in_=xT_d[0, :, 0:2, :])
            nc.sync.dma_start(out=wk2[0][:, 0:4, :], in_=wkh_d[0, :, 0:4, :])
            nc.sync.dma_start(out=xta[:, 2:4, 0:512], in_=xT_d[0, :, 2:4, :])
            nc.sync.dma_start(out=wk2[0][:, 4:NKC, :], in_=wkh_d[0, :, 4:NKC, :])
            nc.sync.dma_start(out=xta[:, 4:8, 0:512], in_=xT_d[0, :, 4:8, :])
            dma_wq(0, wt0)
            for g in range(2, 4):
                nc.sync.dma_start(
                    out=xta[:, 4 * g : 4 * (g + 1), 0:512],
                    in_=xT_d[0, :, 4 * g : 4 * (g + 1), :],
                )
            nc.sync.dma_start(out=em[:], in_=em_d[:, :])
            nc.sync.dma_start(out=onesb[:], in_=onesb_d[:, :])
            nc.sync.dma_start(out=wvm[:], in_=wvh_d[:, :, :])
            for g in range(4):
                nc.sync.dma_start(
                    out=xta[:, 4 * g : 4 * (g + 1), 512:1024],
                    in_=xT_d[1, :, 4 * g : 4 * (g + 1), :],
                )
            # V ones columns via tiny memsets, off the DMA queue
            for sm in range(8):
                nc.vector.memset(vp[sm][:, :, HD : HD + 1], 1.0)
            # zero the rp middle partitions once: the broadcast matmul's
            # zero stationary rows must not meet NaN garbage on hardware
            for i in range(2):
                nc.vector.memset(rpb[i][0:32, :].bitcast(F32), 0.0)
            # HAM warm-up: the PE's clock gate defaults to 4/8 (1.2 GHz)
            # and needs ~3.4us of sustained busy to open. The first real
            # matmul can't start until x/wk stream in (~12us); burn the
            # wait on junk matmuls over the just-memset zero rows so the
            # first K/Q chains run at 2.4 GHz and the PE never idles
            # through the DMA ramp.
            warm_ps = psq.tile([128, 512], F32, tag="psq", name="psq")
            for i in range(8):
                nc.tensor.matmul(
                    warm_ps[:],
                    rpb[0][0:32, 0:128],
                    rpb[0][0:32, :],
                    start=True,
                    stop=True,
                )

            # K projection half-chain -> kt[m2] seq-half n
            def emit_k_half(m2, n):
                ps = psq.tile([128, 512], F32, tag="psq", name="psq")
                for kc in range(NKC):
                    nc.tensor.matmul(
                        ps[:],
                        wk2[m2][:, kc, :],
                        xt[kc][:, 512 * n : 512 * (n + 1)],
                        start=(kc == 0),
                        stop=(kc == NKC - 1),
                    )
                nc.vector.tensor_copy(kt[m2][:, 512 * n : 512 * (n + 1)], ps[:])

            # V projection, all 4 kv heads at once -> vp[sm] (seq-part
            # layout); N=256 moving halves the matmul count vs per-half
            def emit_v(sm):
                ps = psq.tile([128, 512], F32, tag="psq", name="psq")
                for kc in range(NKC):
                    nc.tensor.matmul(
                        ps[:, 0:256],
                        xt[kc][:, 128 * sm : 128 * (sm + 1)],
                        wvm[:, kc, :],
                        start=(kc == 0),
                        stop=(kc == NKC - 1),
                    )
                nc.vector.tensor_copy(
                    vp[sm][:, :, 0:HD],
                    ps[:, 0:256].rearrange("p (g d) -> p g d", g=4),
                )

            # deferred Q projection: returns the qtj tile plus a list of
            # single-matmul thunks to be drained as PE filler inside the
            # ActE-dense attention stripes
            def deferred_qt(j, wt):
                qtj = qtp.tile([128, S], mdt, tag="qt", name="qt")
                state = {}
                thunks = []
                for n in range(2):
                    for kc in range(NKC):
                        def th(n=n, kc=kc):
                            if kc == 0:
                                state[n] = psq.tile(
                                    [128, 512], F32, tag="psq", name="psq"
                                )
                            nc.tensor.matmul(
                                state[n][:],
                                wt[:, kc, :],
                                xt[kc][:, 512 * n : 512 * (n + 1)],
                                start=(kc == 0),
                                stop=(kc == NKC - 1),
                            )
                            if kc == NKC - 1:
                                # split the two chain copies across ActE/DVE
                                # to balance the attention-phase load
                                if n == 0:
                                    nc.scalar.activation(
                                        qtj[:, 0:512],
                                        state[n][:],
                                        mybir.ActivationFunctionType.Copy,
                                    )
                                else:
                                    nc.vector.tensor_copy(
                                        qtj[:, 512:1024], state[n][:]
                                    )
                        thunks.append(th)
                return qtj, thunks

            fq = []

            def drain(k):
                for _ in range(min(k, len(fq))):
                    fq.pop(0)()

            def drain_all():
                while fq:
                    fq.pop(0)()

            def emit_norm_recip(j, qc, pvp):
                """ln(l) rows for the previous stripe on ActE. DVE's
                RECIPROCAL is an iterative 8-cycle/element op (3.4us per
                row) that was stalling the PE at every stripe boundary;
                ln/exp live in one act table set (natural_log_exp_and_
                others) so 1/l = exp(-ln(l)) runs at 1 elem/cycle with no
                table switch."""
                rp = rpb[(2 * j + qc) % 2]
                for h in range(2):
                    with nc.allow_low_precision(reason="f32r is fp32-width"):
                        nc.scalar.activation(
                            rp[32 * h : 32 * h + 1, :],
                            pvp[h][HD : HD + 1, :],
                            mybir.ActivationFunctionType.Ln,
                        )
                return rp

            def emit_norm(j, qc, pvp, rp=None, copies_done=False):
                """Deferred softmax normalize for (j, qc): partition-broadcast
                of the ln(l) rows with a ones matmul, exp(-x) on ActE turns
                the broadcast into 1/l, DVE scales resident O^T. Emitted a
                few score steps into the NEXT stripe so it never stalls the
                PE."""
                if rp is None:
                    rp = emit_norm_recip(j, qc, pvp)
                # one block-diag-ones matmul broadcasts both heads' ln(l)
                # rows; the stationary's zero rows null the uninitialized
                # middle partitions of rp
                bc = psq.tile([128, 512], F32, tag="psq", name="psq")
                nc.tensor.matmul(bc[:], onesb[:], rp[:], start=True, stop=True)
                bcs = bcsp.tile([128, 512], F32, tag="bcs", name="bcs")
                nc.scalar.activation(bcs[:], bc[:], EXPF, scale=-1.0)
                for h in range(2):
                    dst = ot[j][64 * h : 64 * h + 64, 512 * qc : 512 * (qc + 1)]
                    if not copies_done:
                        nc.vector.tensor_copy(dst, pvp[h][0:HD, :])
                    nc.vector.tensor_mul(dst, dst, bcs[64 * h : 64 * h + 64, :])

            def emit_att(j, qtj, qc, pending):
                """Scores+exp+mask+PV for (j, qc); returns (j, qc, pvp) for
                deferred normalization. `pending` is the previous stripe's
                deferral: its O^T copies + ln(l) run behind this stripe's
                first exp (freeing its PSUM), the broadcast+scale a couple
                of steps later."""
                ktj = kt[j // 4]
                kv_even = 2 * (j // 4)
                kbl = [kb for kb in range(NKB) if 128 * kb < 512 * (qc + 1)]
                pvp = [
                    pvs.tile([HD + 1, 512], F32, tag="pvs", name="pvs")
                    for _ in range(2)
                ]
                rp_pend = None
                pts = {}
                for step in range(len(kbl) + LOOK):
                    if step < len(kbl):
                        kb = kbl[step]
                        w = _win(qc, kb)
                        F = 512 - w
                        ps = pssc.tile([128, 1024], F32, tag="pssc", name="pssc")
                        for h in range(2):
                            # per-head windows bank-aligned at 512h (a matmul
                            # output may not cross a PSUM bank boundary)
                            nc.tensor.matmul(
                                ps[:, 512 * h : 512 * h + F],
                                ktj[64 * h : 64 * h + 64, 128 * kb : 128 * (kb + 1)],
                                qtj[64 * h : 64 * h + 64, 512 * qc + w : 512 * (qc + 1)],
                                start=True,
                                stop=True,
                            )
                        pt = ptp.tile([128, 1024], mdt, tag="pt", name="pt")
                        if F == 512:
                            nc.scalar.activation(pt[:], ps[:], EXPF)
                        else:
                            nc.scalar.activation(
                                pt[:].rearrange("p (t q) -> p t q", t=2)[:, :, 0:F],
                                ps[:].rearrange("p (t q) -> p t q", t=2)[:, :, 0:F],
                                EXPF,
                            )
                        if 128 * kb >= 512 * qc:
                            # diagonal sub-block: first 128 cols of window
                            for h in range(2):
                                nc.vector.tensor_mul(
                                    pt[:, 512 * h : 512 * h + 128],
                                    pt[:, 512 * h : 512 * h + 128],
                                    em[:],
                                )
                        pts[kb] = (pt, w, F)
                    if step == 0 and pending is not None:
                        # stage the previous stripe's O^T to SBUF now (frees
                        # its PV PSUM tiles before this stripe's chains need
                        # the pool) and queue ln(l) behind this stripe's
                        # first exp on ActE
                        pj, pqc, ppvp = pending
                        for h in range(2):
                            nc.vector.tensor_copy(
                                ot[pj][64 * h : 64 * h + 64, 512 * pqc : 512 * (pqc + 1)],
                                ppvp[h][0:HD, :],
                            )
                        rp_pend = emit_norm_recip(pj, pqc, ppvp)
                    if step == LOOK + 1 and pending is not None:
                        emit_norm(*pending, rp=rp_pend, copies_done=True)
                        pending = None
                    if step >= LOOK:
                        kb = kbl[step - LOOK]
                        first = step - LOOK == 0
                        last = step - LOOK == len(kbl) - 1
                        pt, w, F = pts.pop(kb)
                        for h in range(2):
                            nc.tensor.matmul(
                                pvp[h][:, w:512],
                                vp[kb][:, kv_even + h, :],
                                pt[:, 512 * h : 512 * h + F],
                                start=first,
                                stop=last,
                            )
                    drain(3)
                if pending is not None:
                    emit_norm(*pending)
                return (j, qc, pvp)

            # emission order: only what attention j0 needs goes first, so
            # the ScalarE exp stream starts as early as possible
            # startup: K/Q seq-half-0 chains interleaved per x-chunk so the
            # PE tracks the x DMA stream; the V chains, remaining halves,
            # and the NEXT pair's Q chain drain as filler inside the
            # ActE-dense attention stripes
            qtj0, qthunks0 = deferred_qt(0, wt0)
            psK = psq.tile([128, 512], F32, tag="psq", name="psq")
            for kc in range(NKC):
                nc.tensor.matmul(
                    psK[:],
                    wk2[0][:, kc, :],
                    xt[kc][:, 0:512],
                    start=(kc == 0),
                    stop=(kc == NKC - 1),
                )
            nc.vector.tensor_copy(kt[0][:, 0:512], psK[:])
            for kc in range(NKC):
                qthunks0[kc]()
            fq.extend([lambda sm=sm: emit_v(sm) for sm in range(4)])
            fq.append(lambda: emit_k_half(0, 1))
            fq.extend(qthunks0[NKC:])

            wot_order = list(range(8))
            qt_cur = qtj0
            pending = None
            for j in range(8):
                if j < 7:
                    wt = wqqp.tile([128, NKC, 128], mdt, tag="wqq", name="wqq")
                    dma_wq(j + 1, wt)
                    qt_next, qthunks = deferred_qt(j + 1, wt)
                    fq.extend(qthunks)
                if j == 0:
                    # second kv-half of wk, behind the wq(1) chunks
                    nc.sync.dma_start(out=wk2[1][:], in_=wkh_d[1])
                if j == 1:
                    # pair-4+ prerequisite: kt[1]
                    fq.extend([lambda n=n: emit_k_half(1, n) for n in range(2)])
                for _ in range(2 if j == 1 else (1 if j >= 2 else 0)):
                    if wot_order:
                        i = wot_order.pop(0)
                        nc.sync.dma_start(
                            out=wot[i][:], in_=woT_d[128 * i : 128 * (i + 1), :]
                        )
                pending = emit_att(j, qt_cur, 0, pending)
                if j == 0:
                    drain_all()
                    for sm in range(4, NSEQ):
                        emit_v(sm)
                pending = emit_att(j, qt_cur, 1, pending)
                drain_all()
                if j < 7:
                    qt_cur = qt_next
            while wot_order:
                i = wot_order.pop(0)
                nc.sync.dma_start(
                    out=wot[i][:], in_=woT_d[128 * i : 128 * (i + 1), :]
                )
            final_norm = pending

        # ---------------- output projection ----------------
        with ExitStack() as ph3:
            outp = ph3.enter_context(tc.tile_pool(name="outsb", bufs=5))
            for dnp in range(2):
                for sm in range(NSEQ):
                    # the last stripe's normalize rides behind the first two
                    # sm chains (which only read qc=0 columns of O^T)
                    if dnp == 0 and sm == 2 and final_norm is not None:
                        emit_norm(*final_norm)
                        final_norm = None
                    # quarter-granular sub-chains only on the very last tile,
                    # so the post-matmul copy+store drain tail is short
                    parts = (
                        [(0, 512), (512, 256), (768, 256)]
                        if (dnp == 1 and sm == NSEQ - 1)
                        else [(0, 512), (512, 512)]
                    )
                    # alternate tiles between the pssc and (otherwise idle)
                    # psq pools so chains never wait on a single pool's
                    # rotation; psq tiles are [128,512] so odd-sm halves each
                    # get their own tile
                    use_psq = sm % 2 == 1 and not (dnp == 1 and sm == NSEQ - 1)
                    if not use_psq:
                        ps = pssc.tile([128, 1024], F32, tag="pssc", name="pssc")
                    for off, fw in parts:
                        if use_psq:
                            half = psq.tile([128, 512], F32, tag="psq", name="psq")
                            dst_ps, dst_off = half, 0
                        else:
                            dst_ps, dst_off = ps, off
                        for qd in range(8):
                            nc.tensor.matmul(
                                dst_ps[:, dst_off : dst_off + fw],
                                ot[qd][:, 128 * sm : 128 * (sm + 1)],
                                wot[qd][
                                    :, 1024 * dnp + off : 1024 * dnp + off + fw
                                ],
                                start=(qd == 0),
                                stop=(qd == 7),
                            )
                        # copy+store each part as soon as its chain stops, so
                        # the drain tail is one part, not a full tile
                        ob = outp.tile([128, 512], F32, tag="outsb", name="outsb")
                        nc.scalar.activation(
                            ob[:, 0:fw],
                            dst_ps[:, dst_off : dst_off + fw],
                            mybir.ActivationFunctionType.Copy,
                        )
                        nc.sync.dma_start(
                            out=out_d[
                                128 * sm : 128 * (sm + 1),
                                1024 * dnp + off : 1024 * dnp + off + fw,
                            ],
                            in_=ob[:, 0:fw],
                        )

    _split_excess_waits(nc)
    nc.finalize()
    return nc


# ---------------------------------------------------------------------------
# host-side preparation
# ---------------------------------------------------------------------------


def _fold_rope(w, cos, sin, nh, scale):
    c = cos[:nh].astype(np.float64)
    s = sin[:nh].astype(np.float64)
    wr = w.astype(np.float64).reshape(nh, HD // 2, 2, w.shape[-1])
    o0 = c[:, :, None] * wr[:, :, 0] - s[:, :, None] * wr[:, :, 1]
    o1 = s[:, :, None] * wr[:, :, 0] + c[:, :, None] * wr[:, :, 1]
    return (np.stack([o0, o1], axis=2).reshape(w.shape) * scale).astype(np.float32)


def _np_dt():
    return mybir.dt.np(_mm_dt())


def kernel(x, freqs_cos, freqs_sin, mask, wq, wk, wv, wo):
    x = np.asarray(x, dtype=np.float32)
    freqs_cos = np.asarray(freqs_cos, dtype=np.float32)
    freqs_sin = np.asarray(freqs_sin, dtype=np.float32)
    mask = np.asarray(mask, dtype=np.float32)
    wq = np.asarray(wq, dtype=np.float32)
    wk = np.asarray(wk, dtype=np.float32)
    wv = np.asarray(wv, dtype=np.float32)
    wo = np.asarray(wo, dtype=np.float32)

    # the kernel hardcodes the causal structure; verify it holds
    causal = np.where(
        np.tril(np.ones((S, S), dtype=bool)), 0.0, -np.inf
    ).astype(np.float32)
    assert np.array_equal(mask, causal), "kernel specialized to causal mask"

    wq_rot = _fold_rope(wq, freqs_cos, freqs_sin, H, 1.0 / np.sqrt(HD))
    wk_rot = _fold_rope(wk, freqs_cos, freqs_sin, KV, 1.0)

    ndt = _np_dt()
    key = os.environ.get("KERNEL_MM_DT", "bf16")
    nc = _module_cache.get(key)
    if nc is None:
        nc = build_module()
        _module_cache[key] = nc

    # S^T layout: tile[k, q] keeps k <= q, i.e. upper-triangular
    em_tril = np.triu(np.ones((128, 128), np.float32)).astype(ndt)
    ones_bd = np.zeros((33, 128), np.float32)
    ones_bd[0, 0:64] = 1.0
    ones_bd[32, 64:128] = 1.0

    in_maps = []
    for c in range(8):
        b, t = divmod(c, 2)
        order = [16 * t + p for p in LOCAL_ORDER]
        kv_heads = list(range(4 * t, 4 * t + 4))
        wq_c = wq_rot.reshape(H, HD, D)[order].reshape(QD, D)
        wk_c = wk_rot.reshape(KV, HD, D)[kv_heads].reshape(KD, D)
        wv_c = wv.reshape(KV, HD, D)[kv_heads].reshape(KD, D)
        wo_c = wo.reshape(D, H, HD)[:, order].reshape(D, QD)
        # packed weight layouts: [chunk-of-128-outputs, 128 D-partitions,
        # NKC D-chunks, 128 outputs], contiguous per chunk for 1-DMA loads
        def pack(wT, nchunk):
            return (
                wT.reshape(NKC, 128, 128 * nchunk)
                .transpose(1, 0, 2)
                .reshape(128, NKC, nchunk, 128)
                .transpose(2, 0, 1, 3)
            )

        wqq = pack(wq_c.T, 8)  # (8, 128, NKC, 128)
        wkh = pack(wk_c.T, 2)  # (2, 128, NKC, 128)
        # wv moving layout: [128 D-partitions, NKC D-chunks, 256 outputs]
        wvh = wv_c.T.reshape(NKC, 128, 2 * 128).transpose(1, 0, 2)
        # x seq-half-major: [half, 128 D-partitions, NKC D-chunks, 512 seq]
        xTh = (
            x[b].T.reshape(NKC, 128, 2, 512).transpose(2, 1, 0, 3)
        )
        in_maps.append(
            {
                "xTh": np.ascontiguousarray(xTh).astype(ndt),
                "wqq": np.ascontiguousarray(wqq).astype(ndt),
                "wkh": np.ascontiguousarray(wkh).astype(ndt),
                "wvh": np.ascontiguousarray(wvh).astype(ndt),
                "woT": np.ascontiguousarray(wo_c.T).astype(ndt),
                "emTril": em_tril,
                "ones_bd": ones_bd,
            }
        )

    trace = bool(os.environ.get("KERNEL_TRACE"))
    res = run_bass_kernel_spmd(nc, in_maps, core_ids=list(range(8)), trace=trace)
    _last_perf["exec_time_ns"] = res.exec_time_ns
    _last_perf["mean_exec_time_ns"] = res.mean_exec_time_ns
    _last_perf["results"] = res

    out = np.empty((B, S, D), np.float32)
    for b in range(B):
        out[b] = res.results[2 * b]["out"] + res.results[2 * b + 1]["out"]
    return out

